# revision 19
# baseline (speedup 1.0000x reference)
"""MLA (multi-head latent) causal attention on 8 Trainium2 NeuronCores.

Sharding: batch(4) x head-group(2) mesh over 8 cores. Core c handles batch
c//2 and heads [8*(c%2), 8*(c%2)+8). The latent KV projections are small and
recomputed per head-group (an MLA property: the latent KV is shared across
heads). Each core produces a partial output (its head-group's contribution to
y @ wo^T for its batch); the host sums the two partials per batch.

Per-core dataflow (all matmuls in float32r - full PE speed, ~1.5e-4 rel rms):
  phase A: q^T = wq_g x^T per head ([D,T] layout, RoPE fused into the
           PSUM->SBUF move via an even/odd-pair row permutation baked into wq
           on the host), latent kv^T = wkv x^T, k^T = wku kv^T (+RoPE),
           v = kv^T-flipped ([T,D] layout).
  phase B: per (head, 512-wide q chunk): scores^T blocks k_j^T q -> causal
           mask on diagonal blocks -> exp on ACT -> row sums via all-ones
           matmul -> fast reciprocal -> y = V^T P unnormalized -> scale ->
           y^T spilled to DRAM scratch.
  phase C: out[t, c] = sum_h y_h^T wo_h^T, PSUM accumulated over heads.
"""

import math
from contextlib import ExitStack

import numpy as np

import concourse.bass as bass
import concourse.mybir as mybir
import concourse.tile as tile
from concourse import bacc
from concourse.bass_utils import run_bass_kernel_spmd

# Problem shape (hardcoded per contract).
B, T, C = 4, 2048, 2048
H, D, L = 16, 128, 512
HG = 8           # heads per core
N_CORES = 8
P = 128
KC = C // P      # 16 contraction chunks over C
LC = L // P      # 4 chunks over L
NQ = T // 512    # 4 query chunks of 512
NT = T // P      # 16 key chunks of 128
SCALE = 1.0 / math.sqrt(D)

F32 = mybir.dt.float32
F32R = mybir.dt.float32r

_cached = {}


def _build_program():
    nc = bacc.Bacc()

    xT = nc.dram_tensor("xT", [C, T], F32R, kind="ExternalInput").ap()
    wqT = nc.dram_tensor("wqT", [C, HG * D], F32R, kind="ExternalInput").ap()
    wkvT = nc.dram_tensor("wkvT", [C, L], F32R, kind="ExternalInput").ap()
    wkuT = nc.dram_tensor("wkuT", [L, D], F32R, kind="ExternalInput").ap()
    wvuT = nc.dram_tensor("wvuT", [L, D], F32R, kind="ExternalInput").ap()
    woT = nc.dram_tensor("woT", [HG * D, C], F32R, kind="ExternalInput").ap()
    c2 = nc.dram_tensor("c2", [P, T], F32, kind="ExternalInput").ap()
    s2 = nc.dram_tensor("s2", [P, T], F32, kind="ExternalInput").ap()
    outp = nc.dram_tensor("outp", [T, C], F32, kind="ExternalOutput").ap()
    ydram = nc.dram_tensor("ydram", [HG, NQ, P, 512], F32R, kind="Internal").ap()
    qdram = nc.dram_tensor("qdram", [HG, NQ, P, 512], F32R, kind="Internal").ap()

    xT_r = xT.rearrange("(kc p) t -> p kc t", p=P)
    wqT_r = wqT.rearrange("(kc p) e -> p kc e", p=P)
    wkvT_r = wkvT.rearrange("(kc p) l -> p kc l", p=P)
    wkuT_r = wkuT.rearrange("(lc p) d -> p lc d", p=P)
    wvuT_r = wvuT.rearrange("(lc p) d -> p lc d", p=P)
    woT_r = woT.rearrange("(h p) c -> p h c", p=P)

    with tile.TileContext(nc) as tc, ExitStack() as top:
        persist = top.enter_context(tc.tile_pool(name="persist", bufs=1))
        # one psum pool shared by all phases: slot-level (not pool-level)
        # reuse avoids cross-phase drain barriers
        pall = top.enter_context(tc.tile_pool(name="pall", bufs=8, space="PSUM"))
        bq = top.enter_context(tc.tile_pool(name="bq", bufs=3))
        k_slab = persist.tile([P, NT, P], F32R)      # k_rot^T: [d, ts_chunk, ts]
        v_slab = persist.tile([P, NT, P], F32R)      # v: [ts, ts_chunk, d]
        tri = persist.tile([P, P], F32)              # additive causal triangle
        ones = persist.tile([P, P], F32R)

        ones_f = persist.tile([P, 1], F32)
        nc.vector.memset(ones_f[:], 1.0)
        nc.vector.tensor_copy(ones[:], ones_f[:].to_broadcast([P, P]))
        nc.gpsimd.memset(tri[:], 0.0)
        nc.gpsimd.affine_select(
            out=tri[:], in_=tri[:],
            compare_op=mybir.AluOpType.is_ge,
            fill=-1e9, base=0,
            pattern=[[1, P]], channel_multiplier=-1,
        )

        # ---------------- phase A: projections + RoPE ----------------
        with ExitStack() as pa:
            xpool = pa.enter_context(tc.tile_pool(name="xpool", bufs=2))
            wpool = pa.enter_context(tc.tile_pool(name="wpool", bufs=6))
            kvpool = pa.enter_context(tc.tile_pool(name="kvpool", bufs=2))
            cspool = pa.enter_context(tc.tile_pool(name="cspool", bufs=2))
            rpool = pa.enter_context(tc.tile_pool(name="rpool", bufs=2))
            single = pa.enter_context(tc.tile_pool(name="single", bufs=1))

            wku_sb = single.tile([P, LC, D], F32R)
            wvu_sb = single.tile([P, LC, D], F32R)
            nc.sync.dma_start(wku_sb[:], wkuT_r)
            nc.sync.dma_start(wvu_sb[:], wvuT_r)
            # wq fully resident (8MB); loaded once, reused by all 4 T chunks.
            # DMA'd in 4 column slices AFTER chunk 0's x/wkv loads are queued
            # (emitted below) so the startup kv matmuls aren't starved.
            wq_sb = single.tile([P, KC, HG * P], F32R)
            wq_loaded = False

            def rope(dst, ps, c2n, s2n):
                # dst = qq * c2n + swap64(qq) * s2n   (all [128, 512])
                # single PSUM read (ACT copy) so the bank frees immediately;
                # the rest runs from SBUF
                qq = rpool.tile([P, 512], F32, tag="qq")
                qs = rpool.tile([P, 512], F32, tag="qs")
                m1 = rpool.tile([P, 512], F32, tag="m1")
                nc.scalar.copy(qq[:], ps[:])
                nc.vector.tensor_copy(qs[0:64, :], qq[64:128, :])
                nc.vector.tensor_copy(qs[64:128, :], qq[0:64, :])
                nc.vector.tensor_tensor(m1[:], qq[:], c2n[:], mybir.AluOpType.mult)
                nc.vector.tensor_tensor(qs[:], qs[:], s2n[:], mybir.AluOpType.mult)
                nc.vector.tensor_tensor(dst, m1[:], qs[:], mybir.AluOpType.add)

            NXG = 4   # xn sub-tiles (kc groups of 4) for rolling release
            for n in range(NQ):
                ts512 = bass.ts(n, 512)
                xns = []
                for g in range(NXG):
                    xg = xpool.tile([P, KC // NXG, 512], F32R, tag=f"xn{g}",
                                    name=f"xn{n}_{g}")
                    if n == 0:
                        for kk in range(KC // NXG):
                            nc.sync.dma_start(
                                xg[:, kk, :],
                                xT_r[:, g * (KC // NXG) + kk, ts512])
                    else:
                        nc.sync.dma_start(
                            xg[:], xT_r[:, bass.ts(g, KC // NXG), ts512])
                    xns.append(xg)

                def xsub(kc):
                    return xns[kc // (KC // NXG)][:, kc % (KC // NXG), :]

                c2n = cspool.tile([P, 512], F32, tag="c2n")
                s2n = cspool.tile([P, 512], F32, tag="s2n")
                nc.sync.dma_start(c2n[:], c2[:, ts512])
                nc.sync.dma_start(s2n[:], s2[:, ts512])

                def emit_q():
                    # q projection: kc-outer from the resident wq; 8 psum banks
                    qps = [pall.tile([P, 512], F32, tag="pa", name=f"qps{n}_{i}")
                           for i in range(HG)]
                    for kc in range(KC):
                        for m in range(HG):
                            nc.tensor.matmul(qps[m][:],
                                             wq_sb[:, kc, bass.ts(m, P)],
                                             xsub(kc),
                                             start=(kc == 0), stop=(kc == KC - 1))
                    for m in range(HG):
                        qst = rpool.tile([P, 512], F32R, tag="qst")
                        rope(qst[:], qps[m][:], c2n, s2n)
                        nc.sync.dma_start(qdram[m, n], qst[:])

                def emit_kv():
                    # latent kv: kc-outer, one streamed weight tile serves all 4
                    # latent chunks (4 psum banks accumulate in parallel)
                    kvps = [pall.tile([P, 512], F32, tag="pa", name=f"kvps{n}_{i}")
                            for i in range(LC)]
                    for kc in range(KC):
                        wkv_t = wpool.tile([P, L], F32R, tag="wkv")
                        nc.sync.dma_start(wkv_t[:], wkvT_r[:, kc, :])
                        for lc in range(LC):
                            nc.tensor.matmul(kvps[lc][:], wkv_t[:, bass.ts(lc, P)],
                                             xsub(kc),
                                             start=(kc == 0), stop=(kc == KC - 1))
                    kvn = kvpool.tile([P, LC, 512], F32R, tag="kvn")
                    for lc in range(LC):
                        nc.scalar.copy(kvn[:, lc, :], kvps[lc][:])

                    # k = wku @ kv, rope, into k_slab
                    kp = pall.tile([P, 512], F32, tag="pa")
                    for lc in range(LC):
                        nc.tensor.matmul(kp[:], wku_sb[:, lc, :], kvn[:, lc, :],
                                         start=(lc == 0), stop=(lc == LC - 1))
                    kdst = k_slab[:, 4 * n:4 * (n + 1), :].rearrange(
                        "p a b -> p (a b)")
                    rope(kdst, kp, c2n, s2n)

                    # v in [t, d] layout (flipped matmul), per 128-token chunk
                    vps = [pall.tile([P, P], F32, tag="pa", name=f"vps{n}_{i}")
                           for i in range(4)]
                    for i in range(4):
                        for lc in range(LC):
                            nc.tensor.matmul(
                                vps[i][:], kvn[:, lc, bass.ts(i, P)],
                                wvu_sb[:, lc, :],
                                start=(lc == 0), stop=(lc == LC - 1))
                        nc.scalar.copy(v_slab[:, 4 * n + i, :], vps[i][:])

                if n == 0:
                    # chunk 0: kv first so the PE has work while the big
                    # resident-wq DMA completes
                    emit_kv()
                    for kcg in range(4):
                        nc.sync.dma_start(
                            wq_sb[:, bass.ts(kcg, KC // 4), :],
                            wqT_r[:, bass.ts(kcg, KC // 4), :])
                    emit_q()
                else:
                    # later chunks: q first so the rope tail drains during kv
                    emit_q()
                    emit_kv()

        # phase C prefetch pools: opened before B so their SBUF zones reuse
        # phase A space (not B space) and the DMAs overlap phase B compute
        with ExitStack() as pc:
            cearly = pc.enter_context(tc.tile_pool(name="cearly", bufs=1))
            cpool = pc.enter_context(tc.tile_pool(name="cpool", bufs=4))
            wo_sl = []
            for ci in range(4):
                w = cearly.tile([P, HG, 512], F32R, name=f"wo{ci}")
                nc.sync.dma_start(w[:], woT_r[:, :, bass.ts(ci, 512)])
                wo_sl.append(w)

            # ---------------- phase B: causal attention ----------------
            with ExitStack() as pb:
                bpool = pb.enter_context(tc.tile_pool(name="bpool", bufs=2))

                for tq in range(NQ):
                    for h in range(HG):
                        nts = 4 * tq + 4
                        spans = [max(P * j - 512 * tq, 0) for j in range(nts)]

                        q_t = bq.tile([P, 512], F32R, tag="qt")
                        nc.sync.dma_start(q_t[:], qdram[h, tq])

                        exp_sb = bpool.tile([P, NT, 512], F32R, tag="exp")
                        nfull = 4 * tq
                        zacc = None
                        if nfull >= 2:
                            zacc = bpool.tile([P, 512], F32R, tag="zacc")
                        for j in range(nts):
                            g = spans[j]
                            sl = slice(g, 512)
                            scp = pall.tile([P, 512], F32, tag="pa",
                                            name=f"scp{tq}_{h}_{j}")
                            nc.tensor.matmul(
                                scp[:, sl], k_slab[:, j, :], q_t[:, sl],
                                start=True, stop=True)
                            if P * j - 512 * tq >= 0:
                                nc.vector.tensor_tensor(
                                    scp[:, g:g + P], scp[:, g:g + P], tri[:],
                                    mybir.AluOpType.add)
                            nc.scalar.activation(
                                exp_sb[:, j, sl], scp[:, sl],
                                mybir.ActivationFunctionType.Exp, scale=SCALE)
                            # fold full blocks into the DVE z accumulator as
                            # they land (keeps PE on scores/PV); one DVE op
                            # per block so masks never queue behind a chain
                            if zacc is not None and j < nfull:
                                if j == 1:
                                    nc.vector.tensor_tensor(
                                        zacc[:], exp_sb[:, 0, :],
                                        exp_sb[:, 1, :], mybir.AluOpType.add)
                                elif j > 1:
                                    nc.vector.tensor_tensor(
                                        zacc[:], zacc[:], exp_sb[:, j, :],
                                        mybir.AluOpType.add)

                        zp = pall.tile([P, 512], F32, tag="pa",
                                       name=f"zp{tq}_{h}")
                        jstart = nfull if zacc is not None else 0
                        ndiag = nts - jstart
                        for dj in range(ndiag):
                            j = jstart + dj
                            sl = slice(spans[j], 512)
                            nc.tensor.matmul(zp[:, sl], ones[:], exp_sb[:, j, sl],
                                             start=(dj == 0),
                                             stop=(dj == ndiag - 1 and zacc is None))
                        if zacc is not None:
                            nc.tensor.matmul(zp[:], ones[:], zacc[:],
                                             start=False, stop=True)
                        zr = bpool.tile([P, 512], F32, tag="zr")
                        nc.vector.reciprocal_approx_fast(out=zr[:], in_=zp[:])

                        yp = pall.tile([P, 512], F32, tag="pa",
                                       name=f"yp{tq}_{h}")
                        for j in range(nts):
                            sl = slice(spans[j], 512)
                            nc.tensor.matmul(yp[:, sl], v_slab[:, j, :],
                                             exp_sb[:, j, sl],
                                             start=(j == 0), stop=(j == nts - 1))
                        yst = bpool.tile([P, 512], F32R, tag="yst")
                        nc.vector.tensor_tensor(yst[:], yp[:], zr[:],
                                                mybir.AluOpType.mult)
                        nc.sync.dma_start(ydram[h, tq], yst[:])

            # ---------------- phase C: output projection ----------------
            for t16 in range(NT):
                y_sb = cpool.tile([P, HG, P], F32R, tag="ysb")
                nc.sync.dma_start(
                    y_sb[:],
                    ydram[:, t16 // 4, :, bass.ts(t16 % 4, P)].rearrange(
                        "h p t -> p h t"))
                for ci in range(4):
                    ops = pall.tile([P, 512], F32, tag="pa",
                                    name=f"ops{t16}_{ci}")
                    for h in range(HG):
                        nc.tensor.matmul(ops[:], y_sb[:, h, :],
                                         wo_sl[ci][:, h, :],
                                         start=(h == 0), stop=(h == HG - 1))
                    ost = cpool.tile([P, 512], F32, tag="ost")
                    nc.scalar.copy(ost[:], ops[:])
                    nc.sync.dma_start(
                        outp[bass.ts(t16, P), bass.ts(ci, 512)], ost[:])

    nc.finalize()
    return nc


_PERM = np.concatenate([np.arange(0, D, 2), np.arange(1, D, 2)])


def _prep_core_inputs(x, freqs_cos, freqs_sin, wq, wkv_down, wk_up, wv_up, wo):
    cosT = np.ascontiguousarray(freqs_cos.T).astype(np.float32)   # [64, T]
    sinT = np.ascontiguousarray(freqs_sin.T).astype(np.float32)
    c2 = np.concatenate([cosT, cosT], axis=0)                     # [128, T]
    s2 = np.concatenate([-sinT, sinT], axis=0)

    wkvT = np.ascontiguousarray(wkv_down.T)                       # [C, L]
    wkuT = np.ascontiguousarray(wk_up[_PERM, :].T)                # [L, D]
    wvuT = np.ascontiguousarray(wv_up.T)                          # [L, D]

    wq_h = wq.reshape(H, D, C)[:, _PERM, :]                       # perm rows/head

    in_maps = []
    for core in range(N_CORES):
        b, g = core // 2, core % 2
        heads = slice(8 * g, 8 * g + 8)
        wqT_g = np.ascontiguousarray(
            wq_h[heads].reshape(HG * D, C).T)                     # [C, 1024]
        woT_g = np.ascontiguousarray(wo[:, 8 * g * D:(8 * g + 8) * D].T)  # [1024, C]
        xT_b = np.ascontiguousarray(x[b].T)                       # [C, T]
        in_maps.append({
            "xT": xT_b, "wqT": wqT_g, "wkvT": wkvT, "wkuT": wkuT,
            "wvuT": wvuT, "woT": woT_g, "c2": c2, "s2": s2,
        })
    return in_maps


def kernel(x, freqs_cos, freqs_sin, wq, wkv_down, wk_up, wv_up, wo, _trace=False):
    x = np.asarray(x, dtype=np.float32)
    freqs_cos = np.asarray(freqs_cos, dtype=np.float32)
    freqs_sin = np.asarray(freqs_sin, dtype=np.float32)
    wq = np.asarray(wq, dtype=np.float32)
    wkv_down = np.asarray(wkv_down, dtype=np.float32)
    wk_up = np.asarray(wk_up, dtype=np.float32)
    wv_up = np.asarray(wv_up, dtype=np.float32)
    wo = np.asarray(wo, dtype=np.float32)

    if "nc" not in _cached:
        _cached["nc"] = _build_program()
    nc = _cached["nc"]

    in_maps = _prep_core_inputs(x, freqs_cos, freqs_sin, wq, wkv_down,
                                wk_up, wv_up, wo)
    res = run_bass_kernel_spmd(nc, in_maps, core_ids=list(range(N_CORES)),
                               trace=_trace)
    _cached["last_result"] = res

    out = np.empty((B, T, C), dtype=np.float32)
    for b in range(B):
        out[b] = res.results[2 * b]["outp"] + res.results[2 * b + 1]["outp"]
    return out


# revision 20
# speedup vs baseline: 1.2281x; 1.2281x over previous
"""MLA (multi-head latent) causal attention on 8 Trainium2 NeuronCores.

Sharding: batch(4) x head-group(2) mesh over 8 cores. Core c handles batch
c//2 and heads [8*(c%2), 8*(c%2)+8). The latent KV projections are small and
recomputed per head-group (an MLA property: the latent KV is shared across
heads). Each core produces a partial output (its head-group's contribution to
y @ wo^T for its batch); the host sums the two partials per batch.

Per-core dataflow (all matmuls in float32r - full PE speed, ~1.5e-4 rel rms):
  phase A: q^T = wq_g x^T per head ([D,T] layout, RoPE fused into the
           PSUM->SBUF move via an even/odd-pair row permutation baked into wq
           on the host), latent kv^T = wkv x^T, k^T = wku kv^T (+RoPE),
           v = kv^T-flipped ([T,D] layout).
  phase B: per (head, 512-wide q chunk): scores^T blocks k_j^T q -> causal
           mask on diagonal blocks -> exp on ACT -> row sums via all-ones
           matmul -> fast reciprocal -> y = V^T P unnormalized -> scale ->
           y^T spilled to DRAM scratch.
  phase C: out[t, c] = sum_h y_h^T wo_h^T, PSUM accumulated over heads.
"""

import math
from contextlib import ExitStack

import numpy as np

import concourse.bass as bass
import concourse.mybir as mybir
import concourse.tile as tile
from concourse import bacc
from concourse.bass_utils import run_bass_kernel_spmd

# Problem shape (hardcoded per contract).
B, T, C = 4, 2048, 2048
H, D, L = 16, 128, 512
HG = 8           # heads per core
N_CORES = 8
P = 128
KC = C // P      # 16 contraction chunks over C
LC = L // P      # 4 chunks over L
NQ = T // 512    # 4 query chunks of 512
NT = T // P      # 16 key chunks of 128
SCALE = 1.0 / math.sqrt(D)

F32 = mybir.dt.float32
F32R = mybir.dt.float32r

_cached = {}


def _build_program():
    nc = bacc.Bacc()

    xT = nc.dram_tensor("xT", [C, T], F32R, kind="ExternalInput").ap()
    wqT = nc.dram_tensor("wqT", [C, HG * D], F32R, kind="ExternalInput").ap()
    wkvT = nc.dram_tensor("wkvT", [C, L], F32R, kind="ExternalInput").ap()
    wkuT = nc.dram_tensor("wkuT", [L, D], F32R, kind="ExternalInput").ap()
    wvuT = nc.dram_tensor("wvuT", [L, D], F32R, kind="ExternalInput").ap()
    woT = nc.dram_tensor("woT", [HG * D, C], F32R, kind="ExternalInput").ap()
    c2 = nc.dram_tensor("c2", [P, T], F32, kind="ExternalInput").ap()
    s2 = nc.dram_tensor("s2", [P, T], F32, kind="ExternalInput").ap()
    outp = nc.dram_tensor("outp", [T, C], F32, kind="ExternalOutput").ap()
    ydram = nc.dram_tensor("ydram", [HG, NQ, P, 512], F32R, kind="Internal").ap()
    qdram = nc.dram_tensor("qdram", [HG, NQ, P, 512], F32R, kind="Internal").ap()

    xT_r = xT.rearrange("(kc p) t -> p kc t", p=P)
    wqT_r = wqT.rearrange("(kc p) e -> p kc e", p=P)
    wkvT_r = wkvT.rearrange("(kc p) l -> p kc l", p=P)
    wkuT_r = wkuT.rearrange("(lc p) d -> p lc d", p=P)
    wvuT_r = wvuT.rearrange("(lc p) d -> p lc d", p=P)
    woT_r = woT.rearrange("(h p) c -> p h c", p=P)

    with tile.TileContext(nc) as tc, ExitStack() as top:
        persist = top.enter_context(tc.tile_pool(name="persist", bufs=1))
        # one psum pool shared by all phases: slot-level (not pool-level)
        # reuse avoids cross-phase drain barriers
        pall = top.enter_context(tc.tile_pool(name="pall", bufs=8, space="PSUM"))
        bq = top.enter_context(tc.tile_pool(name="bq", bufs=3))
        k_slab = persist.tile([P, NT, P], F32R)      # k_rot^T: [d, ts_chunk, ts]
        v_slab = persist.tile([P, NT, P], F32R)      # v: [ts, ts_chunk, d]
        tri = persist.tile([P, P], F32)              # additive causal triangle
        ones = persist.tile([P, P], F32R)

        ones_f = persist.tile([P, 1], F32)
        nc.vector.memset(ones_f[:], 1.0)
        nc.vector.tensor_copy(ones[:], ones_f[:].to_broadcast([P, P]))
        nc.gpsimd.memset(tri[:], 0.0)
        nc.gpsimd.affine_select(
            out=tri[:], in_=tri[:],
            compare_op=mybir.AluOpType.is_ge,
            fill=-1e9, base=0,
            pattern=[[1, P]], channel_multiplier=-1,
        )

        # ---------------- phase A: projections + RoPE ----------------
        with ExitStack() as pa:
            xpool = pa.enter_context(tc.tile_pool(name="xpool", bufs=2))
            wpool = pa.enter_context(tc.tile_pool(name="wpool", bufs=6))
            kvpool = pa.enter_context(tc.tile_pool(name="kvpool", bufs=2))
            cspool = pa.enter_context(tc.tile_pool(name="cspool", bufs=2))
            rpool = pa.enter_context(tc.tile_pool(name="rpool", bufs=2))
            single = pa.enter_context(tc.tile_pool(name="single", bufs=1))

            wku_sb = single.tile([P, LC, D], F32R)
            wvu_sb = single.tile([P, LC, D], F32R)
            nc.sync.dma_start(wku_sb[:], wkuT_r)
            nc.sync.dma_start(wvu_sb[:], wvuT_r)
            # wq fully resident (8MB); loaded once, reused by all 4 T chunks.
            # DMA'd in 4 column slices AFTER chunk 0's x/wkv loads are queued
            # (emitted below) so the startup kv matmuls aren't starved.
            wq_sb = single.tile([P, KC, HG * P], F32R)
            wq_loaded = False

            def rope(dst, ps, c2n, s2n):
                # dst = qq * c2n + swap64(qq) * s2n   (all [128, 512])
                # single PSUM read (ACT copy) so the bank frees immediately;
                # the rest runs from SBUF
                qq = rpool.tile([P, 512], F32, tag="qq")
                qs = rpool.tile([P, 512], F32, tag="qs")
                m1 = rpool.tile([P, 512], F32, tag="m1")
                nc.scalar.copy(qq[:], ps[:])
                nc.vector.tensor_copy(qs[0:64, :], qq[64:128, :])
                nc.vector.tensor_copy(qs[64:128, :], qq[0:64, :])
                nc.vector.tensor_tensor(m1[:], qq[:], c2n[:], mybir.AluOpType.mult)
                nc.vector.tensor_tensor(qs[:], qs[:], s2n[:], mybir.AluOpType.mult)
                nc.vector.tensor_tensor(dst, m1[:], qs[:], mybir.AluOpType.add)

            NXG = 4   # xn sub-tiles (kc groups of 4) for rolling release
            for n in range(NQ):
                ts512 = bass.ts(n, 512)
                xns = []
                for g in range(NXG):
                    xg = xpool.tile([P, KC // NXG, 512], F32R, tag=f"xn{g}",
                                    name=f"xn{n}_{g}")
                    if n == 0:
                        for kk in range(KC // NXG):
                            nc.sync.dma_start(
                                xg[:, kk, :],
                                xT_r[:, g * (KC // NXG) + kk, ts512])
                    else:
                        nc.sync.dma_start(
                            xg[:], xT_r[:, bass.ts(g, KC // NXG), ts512])
                    xns.append(xg)

                def xsub(kc):
                    return xns[kc // (KC // NXG)][:, kc % (KC // NXG), :]

                c2n = cspool.tile([P, 512], F32, tag="c2n")
                s2n = cspool.tile([P, 512], F32, tag="s2n")
                nc.sync.dma_start(c2n[:], c2[:, ts512])
                nc.sync.dma_start(s2n[:], s2[:, ts512])

                def emit_q():
                    # q projection: kc-outer from the resident wq; 8 psum banks
                    qps = [pall.tile([P, 512], F32, tag="pa", name=f"qps{n}_{i}")
                           for i in range(HG)]
                    for kc in range(KC):
                        for m in range(HG):
                            nc.tensor.matmul(qps[m][:],
                                             wq_sb[:, kc, bass.ts(m, P)],
                                             xsub(kc),
                                             start=(kc == 0), stop=(kc == KC - 1))
                    for m in range(HG):
                        qst = rpool.tile([P, 512], F32R, tag="qst")
                        rope(qst[:], qps[m][:], c2n, s2n)
                        nc.sync.dma_start(qdram[m, n], qst[:])

                def emit_kv():
                    # latent kv: kc-outer, one streamed weight tile serves all 4
                    # latent chunks (4 psum banks accumulate in parallel)
                    kvps = [pall.tile([P, 512], F32, tag="pa", name=f"kvps{n}_{i}")
                            for i in range(LC)]
                    for kc in range(KC):
                        wkv_t = wpool.tile([P, L], F32R, tag="wkv")
                        nc.sync.dma_start(wkv_t[:], wkvT_r[:, kc, :])
                        for lc in range(LC):
                            nc.tensor.matmul(kvps[lc][:], wkv_t[:, bass.ts(lc, P)],
                                             xsub(kc),
                                             start=(kc == 0), stop=(kc == KC - 1))
                    kvn = kvpool.tile([P, LC, 512], F32R, tag="kvn")
                    for lc in range(LC):
                        nc.scalar.copy(kvn[:, lc, :], kvps[lc][:])

                    # k = wku @ kv, rope, into k_slab
                    kp = pall.tile([P, 512], F32, tag="pa")
                    for lc in range(LC):
                        nc.tensor.matmul(kp[:], wku_sb[:, lc, :], kvn[:, lc, :],
                                         start=(lc == 0), stop=(lc == LC - 1))
                    kdst = k_slab[:, 4 * n:4 * (n + 1), :].rearrange(
                        "p a b -> p (a b)")
                    rope(kdst, kp, c2n, s2n)

                    # v in [t, d] layout (flipped matmul), per 128-token chunk
                    vps = [pall.tile([P, P], F32, tag="pa", name=f"vps{n}_{i}")
                           for i in range(4)]
                    for i in range(4):
                        for lc in range(LC):
                            nc.tensor.matmul(
                                vps[i][:], kvn[:, lc, bass.ts(i, P)],
                                wvu_sb[:, lc, :],
                                start=(lc == 0), stop=(lc == LC - 1))
                        nc.scalar.copy(v_slab[:, 4 * n + i, :], vps[i][:])

                if n == 0:
                    # chunk 0: kv first so the PE has work while the big
                    # resident-wq DMA completes
                    emit_kv()
                    for kcg in range(4):
                        nc.sync.dma_start(
                            wq_sb[:, bass.ts(kcg, KC // 4), :],
                            wqT_r[:, bass.ts(kcg, KC // 4), :])
                    emit_q()
                else:
                    # later chunks: q first so the rope tail drains during kv
                    emit_q()
                    emit_kv()

        # phase C prefetch pools: opened before B so their SBUF zones reuse
        # phase A space (not B space) and the DMAs overlap phase B compute
        with ExitStack() as pc:
            cearly = pc.enter_context(tc.tile_pool(name="cearly", bufs=1))
            cpool = pc.enter_context(tc.tile_pool(name="cpool", bufs=4))
            wo_sl = []
            for ci in range(4):
                w = cearly.tile([P, HG, 512], F32R, name=f"wo{ci}")
                nc.sync.dma_start(w[:], woT_r[:, :, bass.ts(ci, 512)])
                wo_sl.append(w)

            # ---------------- phase B: causal attention ----------------
            with ExitStack() as pb:
                bpool = pb.enter_context(tc.tile_pool(name="bpool", bufs=2))

                for tq in range(NQ):
                    for h in range(HG):
                        nts = 4 * tq + 4
                        spans = [max(P * j - 512 * tq, 0) for j in range(nts)]

                        q_t = bq.tile([P, 512], F32R, tag="qt")
                        nc.sync.dma_start(q_t[:], qdram[h, tq])

                        exp_sb = bpool.tile([P, NT, 512], F32R, tag="exp")
                        for j in range(nts):
                            g = spans[j]
                            sl = slice(g, 512)
                            scp = pall.tile([P, 512], F32, tag="pa",
                                            name=f"scp{tq}_{h}_{j}")
                            nc.tensor.matmul(
                                scp[:, sl], k_slab[:, j, :], q_t[:, sl],
                                start=True, stop=True)
                            if P * j - 512 * tq >= 0:
                                nc.vector.tensor_tensor(
                                    scp[:, g:g + P], scp[:, g:g + P], tri[:],
                                    mybir.AluOpType.add)
                            nc.scalar.activation(
                                exp_sb[:, j, sl], scp[:, sl],
                                mybir.ActivationFunctionType.Exp, scale=SCALE)

                        zp = pall.tile([P, 512], F32, tag="pa",
                                       name=f"zp{tq}_{h}")
                        for j in range(nts):
                            sl = slice(spans[j], 512)
                            nc.tensor.matmul(zp[:, sl], ones[:], exp_sb[:, j, sl],
                                             start=(j == 0), stop=(j == nts - 1))
                        zr = bpool.tile([P, 512], F32, tag="zr")
                        nc.vector.reciprocal_approx_fast(out=zr[:], in_=zp[:])

                        yp = pall.tile([P, 512], F32, tag="pa",
                                       name=f"yp{tq}_{h}")
                        for j in range(nts):
                            sl = slice(spans[j], 512)
                            nc.tensor.matmul(yp[:, sl], v_slab[:, j, :],
                                             exp_sb[:, j, sl],
                                             start=(j == 0), stop=(j == nts - 1))
                        yst = bpool.tile([P, 512], F32R, tag="yst")
                        nc.vector.tensor_tensor(yst[:], yp[:], zr[:],
                                                mybir.AluOpType.mult)
                        nc.sync.dma_start(ydram[h, tq], yst[:])

            # ---------------- phase C: output projection ----------------
            for t16 in range(NT):
                y_sb = cpool.tile([P, HG, P], F32R, tag="ysb")
                nc.sync.dma_start(
                    y_sb[:],
                    ydram[:, t16 // 4, :, bass.ts(t16 % 4, P)].rearrange(
                        "h p t -> p h t"))
                for ci in range(4):
                    ops = pall.tile([P, 512], F32, tag="pa",
                                    name=f"ops{t16}_{ci}")
                    for h in range(HG):
                        nc.tensor.matmul(ops[:], y_sb[:, h, :],
                                         wo_sl[ci][:, h, :],
                                         start=(h == 0), stop=(h == HG - 1))
                    ost = cpool.tile([P, 512], F32, tag="ost")
                    nc.scalar.copy(ost[:], ops[:])
                    nc.sync.dma_start(
                        outp[bass.ts(t16, P), bass.ts(ci, 512)], ost[:])

    nc.finalize()
    return nc


_PERM = np.concatenate([np.arange(0, D, 2), np.arange(1, D, 2)])


def _prep_core_inputs(x, freqs_cos, freqs_sin, wq, wkv_down, wk_up, wv_up, wo):
    cosT = np.ascontiguousarray(freqs_cos.T).astype(np.float32)   # [64, T]
    sinT = np.ascontiguousarray(freqs_sin.T).astype(np.float32)
    c2 = np.concatenate([cosT, cosT], axis=0)                     # [128, T]
    s2 = np.concatenate([-sinT, sinT], axis=0)

    wkvT = np.ascontiguousarray(wkv_down.T)                       # [C, L]
    wkuT = np.ascontiguousarray(wk_up[_PERM, :].T)                # [L, D]
    wvuT = np.ascontiguousarray(wv_up.T)                          # [L, D]

    wq_h = wq.reshape(H, D, C)[:, _PERM, :]                       # perm rows/head

    in_maps = []
    for core in range(N_CORES):
        b, g = core // 2, core % 2
        heads = slice(8 * g, 8 * g + 8)
        wqT_g = np.ascontiguousarray(
            wq_h[heads].reshape(HG * D, C).T)                     # [C, 1024]
        woT_g = np.ascontiguousarray(wo[:, 8 * g * D:(8 * g + 8) * D].T)  # [1024, C]
        xT_b = np.ascontiguousarray(x[b].T)                       # [C, T]
        in_maps.append({
            "xT": xT_b, "wqT": wqT_g, "wkvT": wkvT, "wkuT": wkuT,
            "wvuT": wvuT, "woT": woT_g, "c2": c2, "s2": s2,
        })
    return in_maps


def kernel(x, freqs_cos, freqs_sin, wq, wkv_down, wk_up, wv_up, wo, _trace=False):
    x = np.asarray(x, dtype=np.float32)
    freqs_cos = np.asarray(freqs_cos, dtype=np.float32)
    freqs_sin = np.asarray(freqs_sin, dtype=np.float32)
    wq = np.asarray(wq, dtype=np.float32)
    wkv_down = np.asarray(wkv_down, dtype=np.float32)
    wk_up = np.asarray(wk_up, dtype=np.float32)
    wv_up = np.asarray(wv_up, dtype=np.float32)
    wo = np.asarray(wo, dtype=np.float32)

    if "nc" not in _cached:
        _cached["nc"] = _build_program()
    nc = _cached["nc"]

    in_maps = _prep_core_inputs(x, freqs_cos, freqs_sin, wq, wkv_down,
                                wk_up, wv_up, wo)
    res = run_bass_kernel_spmd(nc, in_maps, core_ids=list(range(N_CORES)),
                               trace=_trace)
    _cached["last_result"] = res

    out = np.empty((B, T, C), dtype=np.float32)
    for b in range(B):
        out[b] = res.results[2 * b]["outp"] + res.results[2 * b + 1]["outp"]
    return out


# revision 21
# speedup vs baseline: 1.2373x; 1.0075x over previous
"""MLA (multi-head latent) causal attention on 8 Trainium2 NeuronCores.

Sharding: batch(4) x head-group(2) mesh over 8 cores. Core c handles batch
c//2 and heads [8*(c%2), 8*(c%2)+8). The latent KV projections are small and
recomputed per head-group (an MLA property: the latent KV is shared across
heads). Each core produces a partial output (its head-group's contribution to
y @ wo^T for its batch); the host sums the two partials per batch.

Per-core dataflow (all matmuls in float32r - full PE speed, ~1.5e-4 rel rms):
  phase A: q^T = wq_g x^T per head ([D,T] layout, RoPE fused into the
           PSUM->SBUF move via an even/odd-pair row permutation baked into wq
           on the host), latent kv^T = wkv x^T, k^T = wku kv^T (+RoPE),
           v = kv^T-flipped ([T,D] layout).
  phase B: per (head, 512-wide q chunk): scores^T blocks k_j^T q -> causal
           mask on diagonal blocks -> exp on ACT -> row sums via all-ones
           matmul -> fast reciprocal -> y = V^T P unnormalized -> scale ->
           y^T spilled to DRAM scratch.
  phase C: out[t, c] = sum_h y_h^T wo_h^T, PSUM accumulated over heads.
"""

import math
from contextlib import ExitStack

import numpy as np

import concourse.bass as bass
import concourse.mybir as mybir
import concourse.tile as tile
from concourse import bacc
from concourse.bass_utils import run_bass_kernel_spmd

# Problem shape (hardcoded per contract).
B, T, C = 4, 2048, 2048
H, D, L = 16, 128, 512
HG = 8           # heads per core
N_CORES = 8
P = 128
KC = C // P      # 16 contraction chunks over C
LC = L // P      # 4 chunks over L
NQ = T // 512    # 4 query chunks of 512
NT = T // P      # 16 key chunks of 128
SCALE = 1.0 / math.sqrt(D)

F32 = mybir.dt.float32
F32R = mybir.dt.float32r

_cached = {}


def _build_program():
    nc = bacc.Bacc()

    xT = nc.dram_tensor("xT", [C, T], F32R, kind="ExternalInput").ap()
    wqT = nc.dram_tensor("wqT", [C, HG * D], F32R, kind="ExternalInput").ap()
    wkvT = nc.dram_tensor("wkvT", [C, L], F32R, kind="ExternalInput").ap()
    wkuT = nc.dram_tensor("wkuT", [L, D], F32R, kind="ExternalInput").ap()
    wvuT = nc.dram_tensor("wvuT", [L, D], F32R, kind="ExternalInput").ap()
    woT = nc.dram_tensor("woT", [HG * D, C], F32R, kind="ExternalInput").ap()
    c2 = nc.dram_tensor("c2", [P, T], F32, kind="ExternalInput").ap()
    s2 = nc.dram_tensor("s2", [P, T], F32, kind="ExternalInput").ap()
    outp = nc.dram_tensor("outp", [T, C], F32, kind="ExternalOutput").ap()
    ydram = nc.dram_tensor("ydram", [HG, NQ, P, 512], F32R, kind="Internal").ap()
    qdram = nc.dram_tensor("qdram", [HG, NQ, P, 512], F32R, kind="Internal").ap()

    xT_r = xT.rearrange("(kc p) t -> p kc t", p=P)
    wqT_r = wqT.rearrange("(kc p) e -> p kc e", p=P)
    wkvT_r = wkvT.rearrange("(kc p) l -> p kc l", p=P)
    wkuT_r = wkuT.rearrange("(lc p) d -> p lc d", p=P)
    wvuT_r = wvuT.rearrange("(lc p) d -> p lc d", p=P)
    woT_r = woT.rearrange("(h p) c -> p h c", p=P)

    with tile.TileContext(nc) as tc, ExitStack() as top:
        persist = top.enter_context(tc.tile_pool(name="persist", bufs=1))
        # one psum pool shared by all phases: slot-level (not pool-level)
        # reuse avoids cross-phase drain barriers
        pall = top.enter_context(tc.tile_pool(name="pall", bufs=8, space="PSUM"))
        bq = top.enter_context(tc.tile_pool(name="bq", bufs=3))
        k_slab = persist.tile([P, NT, P], F32R)      # k_rot^T: [d, ts_chunk, ts]
        v_slab = persist.tile([P, NT, P], F32R)      # v: [ts, ts_chunk, d]
        tri = persist.tile([P, P], F32)              # additive causal triangle
        ones = persist.tile([P, P], F32R)

        ones_f = persist.tile([P, 1], F32)
        nc.vector.memset(ones_f[:], 1.0)
        nc.vector.tensor_copy(ones[:], ones_f[:].to_broadcast([P, P]))
        nc.gpsimd.memset(tri[:], 0.0)
        nc.gpsimd.affine_select(
            out=tri[:], in_=tri[:],
            compare_op=mybir.AluOpType.is_ge,
            fill=-1e9, base=0,
            pattern=[[1, P]], channel_multiplier=-1,
        )

        # ---------------- phase A: projections + RoPE ----------------
        with ExitStack() as pa:
            xpool = pa.enter_context(tc.tile_pool(name="xpool", bufs=2))
            wpool = pa.enter_context(tc.tile_pool(name="wpool", bufs=6))
            kvpool = pa.enter_context(tc.tile_pool(name="kvpool", bufs=2))
            cspool = pa.enter_context(tc.tile_pool(name="cspool", bufs=2))
            rpool = pa.enter_context(tc.tile_pool(name="rpool", bufs=2))
            single = pa.enter_context(tc.tile_pool(name="single", bufs=1))

            wku_sb = single.tile([P, LC, D], F32R)
            wvu_sb = single.tile([P, LC, D], F32R)
            nc.sync.dma_start(wku_sb[:], wkuT_r)
            nc.sync.dma_start(wvu_sb[:], wvuT_r)
            # wq fully resident (8MB); loaded once, reused by all 4 T chunks.
            # DMA'd in 4 column slices AFTER chunk 0's x/wkv loads are queued
            # (emitted below) so the startup kv matmuls aren't starved.
            wq_sb = single.tile([P, KC, HG * P], F32R)
            wq_loaded = False

            def rope(dst, ps, c2n, s2n):
                # dst = qq * c2n + swap64(qq) * s2n   (all [128, 512])
                # single PSUM read (ACT copy) so the bank frees immediately;
                # the rest runs from SBUF
                qq = rpool.tile([P, 512], F32, tag="qq")
                qs = rpool.tile([P, 512], F32, tag="qs")
                m1 = rpool.tile([P, 512], F32, tag="m1")
                nc.scalar.copy(qq[:], ps[:])
                nc.vector.tensor_copy(qs[0:64, :], qq[64:128, :])
                nc.vector.tensor_copy(qs[64:128, :], qq[0:64, :])
                nc.vector.tensor_tensor(m1[:], qq[:], c2n[:], mybir.AluOpType.mult)
                nc.vector.tensor_tensor(qs[:], qs[:], s2n[:], mybir.AluOpType.mult)
                nc.vector.tensor_tensor(dst, m1[:], qs[:], mybir.AluOpType.add)

            NXG = 4   # xn sub-tiles (kc groups of 4) for rolling release
            for n in range(NQ):
                ts512 = bass.ts(n, 512)
                xns = []
                for g in range(NXG):
                    xg = xpool.tile([P, KC // NXG, 512], F32R, tag=f"xn{g}",
                                    name=f"xn{n}_{g}")
                    if n == 0:
                        for kk in range(KC // NXG):
                            nc.sync.dma_start(
                                xg[:, kk, :],
                                xT_r[:, g * (KC // NXG) + kk, ts512])
                    else:
                        nc.sync.dma_start(
                            xg[:], xT_r[:, bass.ts(g, KC // NXG), ts512])
                    xns.append(xg)

                def xsub(kc):
                    return xns[kc // (KC // NXG)][:, kc % (KC // NXG), :]

                c2n = cspool.tile([P, 512], F32, tag="c2n")
                s2n = cspool.tile([P, 512], F32, tag="s2n")
                nc.sync.dma_start(c2n[:], c2[:, ts512])
                nc.sync.dma_start(s2n[:], s2[:, ts512])

                def emit_q():
                    # q projection: kc-outer from the resident wq; 8 psum banks
                    qps = [pall.tile([P, 512], F32, tag="pa", name=f"qps{n}_{i}")
                           for i in range(HG)]
                    for kc in range(KC):
                        for m in range(HG):
                            nc.tensor.matmul(qps[m][:],
                                             wq_sb[:, kc, bass.ts(m, P)],
                                             xsub(kc),
                                             start=(kc == 0), stop=(kc == KC - 1))
                    for m in range(HG):
                        qst = rpool.tile([P, 512], F32R, tag="qst")
                        rope(qst[:], qps[m][:], c2n, s2n)
                        nc.sync.dma_start(qdram[m, n], qst[:])

                def emit_kv():
                    # latent kv: kc-outer, one streamed weight tile serves all 4
                    # latent chunks (4 psum banks accumulate in parallel)
                    kvps = [pall.tile([P, 512], F32, tag="pa", name=f"kvps{n}_{i}")
                            for i in range(LC)]
                    for kc in range(KC):
                        wkv_t = wpool.tile([P, L], F32R, tag="wkv")
                        nc.sync.dma_start(wkv_t[:], wkvT_r[:, kc, :])
                        for lc in range(LC):
                            nc.tensor.matmul(kvps[lc][:], wkv_t[:, bass.ts(lc, P)],
                                             xsub(kc),
                                             start=(kc == 0), stop=(kc == KC - 1))
                    kvn = kvpool.tile([P, LC, 512], F32R, tag="kvn")
                    for lc in range(LC):
                        nc.scalar.copy(kvn[:, lc, :], kvps[lc][:])

                    # k = wku @ kv, rope, into k_slab
                    kp = pall.tile([P, 512], F32, tag="pa")
                    for lc in range(LC):
                        nc.tensor.matmul(kp[:], wku_sb[:, lc, :], kvn[:, lc, :],
                                         start=(lc == 0), stop=(lc == LC - 1))
                    kdst = k_slab[:, 4 * n:4 * (n + 1), :].rearrange(
                        "p a b -> p (a b)")
                    rope(kdst, kp, c2n, s2n)

                    # v in [t, d] layout (flipped matmul), per 128-token chunk
                    vps = [pall.tile([P, P], F32, tag="pa", name=f"vps{n}_{i}")
                           for i in range(4)]
                    for i in range(4):
                        for lc in range(LC):
                            nc.tensor.matmul(
                                vps[i][:], kvn[:, lc, bass.ts(i, P)],
                                wvu_sb[:, lc, :],
                                start=(lc == 0), stop=(lc == LC - 1))
                        nc.scalar.copy(v_slab[:, 4 * n + i, :], vps[i][:])

                if n == 0:
                    # chunk 0: kv first so the PE has work while the big
                    # resident-wq DMA completes
                    emit_kv()
                    for kcg in range(4):
                        nc.sync.dma_start(
                            wq_sb[:, bass.ts(kcg, KC // 4), :],
                            wqT_r[:, bass.ts(kcg, KC // 4), :])
                    emit_q()
                else:
                    # later chunks: q first so the rope tail drains during kv
                    emit_q()
                    emit_kv()

        # phase C prefetch pools: opened before B so their SBUF zones reuse
        # phase A space (not B space) and the DMAs overlap phase B compute
        with ExitStack() as pc:
            cearly = pc.enter_context(tc.tile_pool(name="cearly", bufs=1))
            cpool = pc.enter_context(tc.tile_pool(name="cpool", bufs=4))
            wo_sl = []
            for ci in range(4):
                w = cearly.tile([P, HG, 512], F32R, name=f"wo{ci}")
                nc.sync.dma_start(w[:], woT_r[:, :, bass.ts(ci, 512)])
                wo_sl.append(w)

            # ---------------- phase B: causal attention ----------------
            with ExitStack() as pb:
                bpool = pb.enter_context(tc.tile_pool(name="bpool", bufs=2))

                def emit_zpv(tq, h, nts, spans, exp_sb):
                    zp = pall.tile([P, 512], F32, tag="pa", name=f"zp{tq}_{h}")
                    for j in range(nts):
                        sl = slice(spans[j], 512)
                        nc.tensor.matmul(zp[:, sl], ones[:], exp_sb[:, j, sl],
                                         start=(j == 0), stop=(j == nts - 1))
                    zr = bpool.tile([P, 512], F32, tag="zr")
                    nc.vector.reciprocal_approx_fast(out=zr[:], in_=zp[:])

                    yp = pall.tile([P, 512], F32, tag="pa", name=f"yp{tq}_{h}")
                    for j in range(nts):
                        sl = slice(spans[j], 512)
                        nc.tensor.matmul(yp[:, sl], v_slab[:, j, :],
                                         exp_sb[:, j, sl],
                                         start=(j == 0), stop=(j == nts - 1))
                    yst = bpool.tile([P, 512], F32R, tag="yst")
                    nc.vector.tensor_tensor(yst[:], yp[:], zr[:],
                                            mybir.AluOpType.mult)
                    nc.sync.dma_start(ydram[h, tq], yst[:])

                # software pipeline: scores/exp of iteration i+1 are emitted
                # before z/PV of iteration i, so the PE has dense work while
                # ACT chews through iteration i+1's exps
                pending = None
                for tq in range(NQ):
                    for h in range(HG):
                        nts = 4 * tq + 4
                        spans = [max(P * j - 512 * tq, 0) for j in range(nts)]

                        q_t = bq.tile([P, 512], F32R, tag="qt")
                        nc.sync.dma_start(q_t[:], qdram[h, tq])

                        exp_sb = bpool.tile([P, NT, 512], F32R, tag="exp")
                        for j in range(nts):
                            g = spans[j]
                            sl = slice(g, 512)
                            scp = pall.tile([P, 512], F32, tag="pa",
                                            name=f"scp{tq}_{h}_{j}")
                            nc.tensor.matmul(
                                scp[:, sl], k_slab[:, j, :], q_t[:, sl],
                                start=True, stop=True)
                            if P * j - 512 * tq >= 0:
                                nc.vector.tensor_tensor(
                                    scp[:, g:g + P], scp[:, g:g + P], tri[:],
                                    mybir.AluOpType.add)
                            nc.scalar.activation(
                                exp_sb[:, j, sl], scp[:, sl],
                                mybir.ActivationFunctionType.Exp, scale=SCALE)

                        if pending is not None:
                            emit_zpv(*pending)
                        pending = (tq, h, nts, spans, exp_sb)
                emit_zpv(*pending)

            # ---------------- phase C: output projection ----------------
            for t16 in range(NT):
                y_sb = cpool.tile([P, HG, P], F32R, tag="ysb")
                nc.sync.dma_start(
                    y_sb[:],
                    ydram[:, t16 // 4, :, bass.ts(t16 % 4, P)].rearrange(
                        "h p t -> p h t"))
                for ci in range(4):
                    ops = pall.tile([P, 512], F32, tag="pa",
                                    name=f"ops{t16}_{ci}")
                    for h in range(HG):
                        nc.tensor.matmul(ops[:], y_sb[:, h, :],
                                         wo_sl[ci][:, h, :],
                                         start=(h == 0), stop=(h == HG - 1))
                    ost = cpool.tile([P, 512], F32, tag="ost")
                    nc.scalar.copy(ost[:], ops[:])
                    nc.sync.dma_start(
                        outp[bass.ts(t16, P), bass.ts(ci, 512)], ost[:])

    nc.finalize()
    return nc


_PERM = np.concatenate([np.arange(0, D, 2), np.arange(1, D, 2)])


def _prep_core_inputs(x, freqs_cos, freqs_sin, wq, wkv_down, wk_up, wv_up, wo):
    cosT = np.ascontiguousarray(freqs_cos.T).astype(np.float32)   # [64, T]
    sinT = np.ascontiguousarray(freqs_sin.T).astype(np.float32)
    c2 = np.concatenate([cosT, cosT], axis=0)                     # [128, T]
    s2 = np.concatenate([-sinT, sinT], axis=0)

    wkvT = np.ascontiguousarray(wkv_down.T)                       # [C, L]
    wkuT = np.ascontiguousarray(wk_up[_PERM, :].T)                # [L, D]
    wvuT = np.ascontiguousarray(wv_up.T)                          # [L, D]

    wq_h = wq.reshape(H, D, C)[:, _PERM, :]                       # perm rows/head

    in_maps = []
    for core in range(N_CORES):
        b, g = core // 2, core % 2
        heads = slice(8 * g, 8 * g + 8)
        wqT_g = np.ascontiguousarray(
            wq_h[heads].reshape(HG * D, C).T)                     # [C, 1024]
        woT_g = np.ascontiguousarray(wo[:, 8 * g * D:(8 * g + 8) * D].T)  # [1024, C]
        xT_b = np.ascontiguousarray(x[b].T)                       # [C, T]
        in_maps.append({
            "xT": xT_b, "wqT": wqT_g, "wkvT": wkvT, "wkuT": wkuT,
            "wvuT": wvuT, "woT": woT_g, "c2": c2, "s2": s2,
        })
    return in_maps


def kernel(x, freqs_cos, freqs_sin, wq, wkv_down, wk_up, wv_up, wo, _trace=False):
    x = np.asarray(x, dtype=np.float32)
    freqs_cos = np.asarray(freqs_cos, dtype=np.float32)
    freqs_sin = np.asarray(freqs_sin, dtype=np.float32)
    wq = np.asarray(wq, dtype=np.float32)
    wkv_down = np.asarray(wkv_down, dtype=np.float32)
    wk_up = np.asarray(wk_up, dtype=np.float32)
    wv_up = np.asarray(wv_up, dtype=np.float32)
    wo = np.asarray(wo, dtype=np.float32)

    if "nc" not in _cached:
        _cached["nc"] = _build_program()
    nc = _cached["nc"]

    in_maps = _prep_core_inputs(x, freqs_cos, freqs_sin, wq, wkv_down,
                                wk_up, wv_up, wo)
    res = run_bass_kernel_spmd(nc, in_maps, core_ids=list(range(N_CORES)),
                               trace=_trace)
    _cached["last_result"] = res

    out = np.empty((B, T, C), dtype=np.float32)
    for b in range(B):
        out[b] = res.results[2 * b]["outp"] + res.results[2 * b + 1]["outp"]
    return out
